# revision 19
# baseline (speedup 1.0000x reference)
"""Trainium2 Bass kernel for nn_BlockShuffleLayer (butterfly block-diag MLP).

Math (reference):
  out1[b, k, q] = sum_p x[b, k*256+p] * w1[k, q, p]          (k=16 blocks, p=q=256)
  shuffle: kq index (k*256+q) viewed as (r, l), r=kq//16, l=kq%16
  out2[b, s, l] = sum_r out1s[b, l, r] * w2[l, s, r]          (l=16 blocks, r=256, s=1024)
  out[b, s*16+l] = out2[b, s, l]

Strategy: data-parallel over the 4096-token batch across 8 cores (512 tokens
each), weights replicated.  Per core, the 512 tokens are processed as two
software-pipelined halves so half 1's stage-1 (input DMA + matmuls) overlaps
half 0's stage-2 (output DMA):

  phase A (stage 1, output feature-major), per half:
    - x arrives host-transposed (xt[p, b]) so the contraction dim is already
      on partitions: zero on-chip transposes and a pure matmul stream on PE
      (keeps the HAM clock warm).
    - stage-1 matmuls produce out1T[q'', b] in PSUM with w1 column-permuted
      on host so that one DVE/ACT copy + one SBUF->SBUF DMA per k-pair
      scatters the butterfly shuffle into the z layout (16-partition
      stripes); z rows come out in natural r order for stage 2.
  phase B (stage 2, tokens-major), per (half, s-half):
    - w2 resident as 16 per-l tiles (streamed in s-halves; per-l tiles let
      the second s-half reload overlap remaining first-half compute)
    - N=512 matmuls z^T @ w2 -> psum[b, s]
    - psum scatter-copied (stride-16 SBUF writes on DVE/ACT) into the
      interleaved output columns, then contiguous 2MB DMAs to DRAM.
"""

import numpy as np

import concourse.bacc as bacc
import concourse.bass as bass
import concourse.mybir as mybir
import concourse.tile as tile
from concourse import bass_utils

FP32 = mybir.dt.float32
# float32r: fp32 operands with single-pass (relaxed-precision) multiply --
# 4x PE throughput vs true fp32; HW-measured rel err ~2e-4 end-to-end
# (TF32-class).  Set to mybir.dt.float32 for exact fp32 at ~2x the runtime.
MMDT = mybir.dt.float32r

K, Q, P = 16, 256, 256
L, S, R = 16, 1024, 256
N_IN = K * P          # 4096
N_OUT = S * L         # 16384
BATCH = 4096
NCORES = 8
SHARD = BATCH // NCORES


def build_kernel(n_tokens: int = SHARD, reps: int = 1,
                 serialize_reps: bool = False,
                 n_halves: int = 2) -> bass.Bass:
    half = n_tokens // n_halves
    nbc_h = half // 128
    assert half % 128 == 0
    nc = bacc.Bacc("TRN2", target_bir_lowering=False, debug=False,
                   num_devices=NCORES)

    # host-prepared layouts (see _prep_weights / kernel):
    #   xt[P, b]                      = x[b, P]  (pre-transposed shard)
    #   w1t[p, k, pc, qc*128+u]       = w1[k, (u%16)*16 + qc*8 + u//16, pc*128+p]
    #   w2t[sh, r', l, rc, s']        = w2[l, sh*512+s', rc*128+r']
    xt = nc.dram_tensor("xt", [N_IN, n_tokens], FP32, kind="ExternalInput")
    w1t = nc.dram_tensor("w1t", [128, K, 2, Q], FP32, kind="ExternalInput")
    w2t = nc.dram_tensor("w2t", [2, 128, L, 2, 512], FP32, kind="ExternalInput")
    out = nc.dram_tensor("out", [n_tokens, N_OUT], FP32, kind="ExternalOutput")

    with tile.TileContext(nc) as tc:
        with tc.tile_pool(name="const", bufs=1) as cpool, \
             tc.tile_pool(name="pa", bufs=1) as pa, \
             tc.tile_pool(name="pap", bufs=3, space="PSUM") as pap, \
             tc.tile_pool(name="pb", bufs=2) as pb, \
             tc.tile_pool(name="pbp", bufs=4, space="PSUM") as pbp:
            # z[u', l, rc, b] per half: shuffled stage-1 out; r = rc*128 + u'
            z_sb = [cpool.tile([128, L, 2, half], MMDT, name=f"z{hv}")
                    for hv in range(n_halves)]
            # w2 s-half as 16 per-l tiles (see module docstring)
            w2h = [cpool.tile([128, 2, 512], MMDT, name=f"w2h{l}")
                   for l in range(L)]

            def phase_a(hv):
                # k-pairs (k0, k0+8): their stripes land in the same 16 z
                # partitions (differing only in the rc slot), so one DMA per
                # (pair, t) scatters 4 stripes at once
                for k0 in range(8):
                    if hv == 0:
                        # prefetch first w2 s-half behind stage-1 compute
                        for l in (2 * k0, 2 * k0 + 1):
                            nc.scalar.dma_start(w2h[l][:],
                                                w2t[0, :, l].bitcast(MMDT))
                    stg = pa.tile([128, 2, 2, half], MMDT, tag="stg",
                                  name="stg", bufs=2)   # [u, qc, kh, b]
                    for kh in range(2):
                        k = k0 + 8 * kh
                        w1k = pa.tile([128, 2, Q], MMDT, tag="w1k",
                                      name="w1k", bufs=3)
                        nc.sync.dma_start(w1k[:], w1t[:, k].bitcast(MMDT))
                        xtk = pa.tile([128, 2, half], MMDT, tag="xtk",
                                      name="xtk", bufs=3)
                        nc.sync.dma_start(
                            xtk[:],
                            xt[k * P:(k + 1) * P,
                               hv * half:(hv + 1) * half].rearrange(
                                "(pc p) b -> p pc b", p=128).bitcast(MMDT))
                        for qc in range(2):
                            ps1 = pap.tile([128, half], FP32,
                                           tag="ps1", name="ps1")
                            for pc in range(2):
                                nc.tensor.matmul(
                                    ps1[:],
                                    w1k[:, pc, qc * 128:(qc + 1) * 128],
                                    xtk[:, pc, :],
                                    start=(pc == 0), stop=(pc == 1))
                            if (kh + qc) % 2 == 0:
                                nc.vector.tensor_copy(stg[:, qc, kh, :],
                                                      ps1[:])
                            else:
                                nc.scalar.copy(stg[:, qc, kh, :], ps1[:])
                    # butterfly redistribution: psum partition u = 16t+j
                    # holds column (l = qc*8+t, j); z row u' = k0*16+j,
                    # rc = kh, so r = rc*128+u' is natural for w2.
                    for t in range(8):
                        nc.sync.dma_start(
                            z_sb[hv][k0 * 16:k0 * 16 + 16, t:t + 9:8, :, :],
                            stg[16 * t:16 * t + 16, :, :, :])

            def phase_b(hv, sh):
                for bc in range(nbc_h):
                    row0 = hv * half + bc * 128
                    obs = [pb.tile([128, L * 256], FP32, tag="ob", name="ob")
                           for _ in range(2)]
                    for l in range(L):
                        ps2 = pbp.tile([128, 512], FP32, tag="ps2",
                                       name="ps2")
                        for rc in range(2):
                            nc.tensor.matmul(
                                ps2[:],
                                z_sb[hv][:, l, rc, bc * 128:(bc + 1) * 128],
                                w2h[l][:, rc, :],
                                start=(rc == 0), stop=(rc == 1))
                        for qq in range(2):
                            ob3 = obs[qq][:].rearrange("p (s l) -> p s l",
                                                       l=L)
                            if (l + qq) % 2 == 0:
                                nc.vector.tensor_copy(
                                    ob3[:, :, l],
                                    ps2[:, qq * 256:(qq + 1) * 256])
                            else:
                                nc.scalar.copy(
                                    ob3[:, :, l],
                                    ps2[:, qq * 256:(qq + 1) * 256])
                    for qq in range(2):
                        c0 = sh * 8192 + qq * 4096
                        nc.sync.dma_start(
                            out[row0:row0 + 128, c0:c0 + 4096], obs[qq][:])

            for _rep in range(reps):
                # software pipeline: A(1) overlaps B(0, s-half 0), the w2
                # second-half reload overlaps B(last, s-half 0), etc.
                for hv in range(n_halves):
                    phase_a(hv)
                for hv in range(n_halves):
                    phase_b(hv, 0)
                for l in range(L):
                    nc.scalar.dma_start(w2h[l][:], w2t[1, :, l].bitcast(MMDT))
                for hv in range(n_halves):
                    phase_b(hv, 1)
                if serialize_reps and _rep != reps - 1:
                    # benchmarking only: forbid cross-rep overlap so the
                    # reps-slope measures a full single-invocation span
                    tc.strict_bb_all_engine_barrier()

    nc.compile()
    return nc


# stage-1 psum chunk qc, partition u = 16t+j holds output column
# q = j*16 + (qc*8 + t)
_QCOL = np.array([(u % 16) * 16 + (qc * 8) + u // 16
                  for qc in range(2) for u in range(128)])


def _prep_weights(w1: np.ndarray, w2: np.ndarray):
    # w1t[p, k, pc, q''] = w1[k, _QCOL[q''], pc*128+p]
    w1p = w1[:, _QCOL, :]                        # [k, q'', P]
    w1t = np.ascontiguousarray(
        w1p.reshape(K, Q, 2, 128).transpose(3, 0, 2, 1))
    # w2t[sh, r', l, rc, s'] = w2[l, sh*512+s', rc*128+r']
    w2t = np.ascontiguousarray(
        w2.reshape(L, 2, 512, 2, 128).transpose(1, 4, 0, 3, 2))
    return w1t, w2t


_NC_CACHE: dict = {}


def kernel(x, w1, w2) -> np.ndarray:
    x = np.asarray(x, dtype=np.float32)
    w1 = np.asarray(w1, dtype=np.float32)
    w2 = np.asarray(w2, dtype=np.float32)
    assert x.shape == (BATCH, N_IN) and w1.shape == (K, Q, P) \
        and w2.shape == (L, S, R)

    if "nc" not in _NC_CACHE:
        _NC_CACHE["nc"] = build_kernel(SHARD)
    nc = _NC_CACHE["nc"]

    w1t, w2t = _prep_weights(w1, w2)
    in_maps = [
        {"xt": np.ascontiguousarray(x[i * SHARD:(i + 1) * SHARD].T),
         "w1t": w1t, "w2t": w2t}
        for i in range(NCORES)
    ]
    res = bass_utils.run_bass_kernel_spmd(nc, in_maps,
                                          core_ids=list(range(NCORES)))
    return np.concatenate([r["out"] for r in res.results], axis=0)
